# revision 1
# baseline (speedup 1.0000x reference)
"""BatchAllTripletLoss on 8 Trainium2 NeuronCores via Bass/Tile.

Math: for anchors i, positives j (same label, j!=i), negatives k (diff label):
  total        = sum_{i,j,k} relu(d_ij - d_ik + margin)
  num_non_easy = #{(i,j,k): d_ik < d_ij + margin}
  loss         = total / num_non_easy ; frac = num_non_easy / num_valid

Strategy. Anchors are grouped into UNITS of up to two anchors from two
different labels whose member counts fit one 128-row k-tile (m1+m2 <= 128).
Each core gets U=52 units (ALOC=104 anchor slots incl. dummies) and its own
column permutation of the 640 samples that places the relevant label blocks
at fixed offsets (side A at row 0, side B at row m1) of k-tiles 0..2; other
samples fill the gaps. Host-built select-masks compact, per unit, the
thresholds t'_p = (dist + margin) of BOTH anchors' positive sets onto the
128 partitions (self-row excluded, everything else 0).

Per unit (v' = sqrt(d2 + BIG*same_label) bf16, broadcast [128,640]):
  - mt = (v' < t') one non-accumulating DVE is_lt (2x bf16 mode), giving
    the 0/1 matrix for BOTH anchors at once.
  - count: PE ones-matmul of mt ACCUMULATED across all units into a single
    [1,640] psum row; one ACT drain at the end -> per-core count scalar.
  - relu-sum: one accumulating pass per unit, DVE stt min(v'-t',0) (negated
    on host) or ACT Relu(t'-v'), statically balanced; per-partition row
    sums land in ACC[:, u], which is DMA'd out and reduced on host.
Distances come from PE alone: f32r dot matmul + rank-1 psum updates add the
squared norms, so no norm-broadcast DMAs. v' rows are staged to DRAM and
DMA-broadcast (stride-0 partition source), four units per load, prefetched.
num_valid is pure label counting (host, exact).
"""

import numpy as np

N = 640
D = 128
NCORES = 8
UNITS = 59                    # units per core (re-derived from labels)
ALOC = 2 * UNITS              # anchor slots per core (2 per unit)
NBLK = 3                      # k-tiles that may hold label blocks
SPLIT = 64                    # partition row where side-B blocks start
MARGIN = 1.9
BIG = 1.0e9
PREFETCH = 2                  # vb group (8 units) prefetch depth

_CACHE = {}


def _relu_engine(u):
    # relu-sum engine per unit slot: ~37% on DVE, rest on ACT
    return "D" if u % 8 in (1, 4, 6) else "A"


def _build_program():
    import concourse.bass as bass
    import concourse.bacc as bacc
    import concourse.mybir as mybir
    import concourse.tile as tile
    from concourse.masks import make_identity

    f32 = mybir.dt.float32
    bf16 = mybir.dt.bfloat16
    Alu = mybir.AluOpType
    Act = mybir.ActivationFunctionType

    nc = bacc.Bacc("TRN2", target_bir_lowering=False, debug=False,
                   num_devices=NCORES)

    # efT columns pre-scaled by -2 on host; elocT unscaled.
    efT2 = nc.declare_dram_parameter("efT2", [D, N], f32, isOutput=False)
    elocT = nc.declare_dram_parameter("elocT", [D, ALOC], f32, isOutput=False)
    labrow = nc.declare_dram_parameter("labrow", [1, N], f32, isOutput=False)
    llocT = nc.declare_dram_parameter("llocT", [ALOC, 1], f32, isOutput=False)
    # select masks: [128, NBLK(tile) * 2(side) * UNITS]
    selm = nc.declare_dram_parameter("selm", [128, NBLK * 2 * UNITS], f32,
                                     isOutput=False)
    out_d = nc.declare_dram_parameter("out", [128, UNITS], f32, isOutput=True)
    out2_d = nc.declare_dram_parameter("out2", [1, 2], f32, isOutput=True)

    with tile.TileContext(nc) as tc:
        with (
            tc.tile_pool(name="singles", bufs=1) as sg,
            tc.tile_pool(name="vbp", bufs=PREFETCH + 2) as vbp,
            tc.tile_pool(name="mtp", bufs=6) as mtp,
            tc.tile_pool(name="junkp", bufs=3) as junkp,
            tc.tile_pool(name="dram", bufs=1, space="DRAM") as dram,
            tc.tile_pool(name="ps_mm", bufs=1, space="PSUM") as ps_mm,
            tc.tile_pool(name="ps_tr", bufs=2, space="PSUM") as ps_tr,
            tc.tile_pool(name="ps_q", bufs=1, space="PSUM") as ps_q,
        ):
            # ---- load inputs ----
            EF2 = sg.tile([D, N], f32)
            nc.gpsimd.dma_start(out=EF2[:], in_=efT2[:])
            EL = sg.tile([D, ALOC], f32)
            nc.gpsimd.dma_start(out=EL[:], in_=elocT[:])
            LLT = sg.tile([ALOC, 1], f32)
            nc.gpsimd.dma_start(out=LLT[:], in_=llocT[:])
            SELM = sg.tile([128, NBLK * 2 * UNITS], f32)
            nc.gpsimd.dma_start(out=SELM[:], in_=selm[:])
            LBC = sg.tile([ALOC, N], f32)
            nc.sync.dma_start(out=LBC[:], in_=labrow[:].to_broadcast([ALOC, N]))

            ident = sg.tile([128, 128], f32)
            make_identity(nc, ident[:])
            ones = sg.tile([128, 1], f32)
            nc.vector.memset(ones[:], 1.0)
            onesb = sg.tile([128, 1], bf16)
            nc.vector.memset(onesb[:], 1.0)
            ONESR = sg.tile([1, N], f32)
            nc.vector.memset(ONESR[:], 1.0)
            ZB = sg.tile([128, N], bf16)
            nc.vector.memset(ZB[:], 0.0)

            # ---- squared distances fully in PSUM ----
            # d2[a,k] = (-2 e_a).e_k + |e|^2[k] + |e_a|^2
            Esq = sg.tile([D, N], f32)
            nc.vector.scalar_tensor_tensor(out=Esq[:], in0=EF2[:], scalar=0.25,
                                           in1=EF2[:],
                                           op0=Alu.mult, op1=Alu.mult)
            ELsq = sg.tile([D, ALOC], f32)
            nc.vector.tensor_mul(ELsq[:], EL[:], EL[:])

            sqf_ps = ps_tr.tile([1, N], f32, tag="tr", name="sqf")
            nc.tensor.matmul(sqf_ps[:, 0:512], ones[:], Esq[:, 0:512])
            nc.tensor.matmul(sqf_ps[:, 512:N], ones[:], Esq[:, 512:N])
            SQF = sg.tile([1, N], f32)
            nc.vector.tensor_copy(SQF[:], sqf_ps[:])

            sql_ps = ps_tr.tile([1, ALOC], f32, tag="tr", name="sql")
            nc.tensor.matmul(sql_ps[:], ones[:], ELsq[:])
            SQLR = sg.tile([1, ALOC], f32)
            nc.vector.tensor_copy(SQLR[:], sql_ps[:])

            dot_ps = ps_mm.tile([ALOC, N], f32, tag="mm", name="dot")
            nc.tensor.matmul(dot_ps[:, 0:512], EL[:],
                             EF2[:, 0:512],
                             start=True, stop=False)
            nc.tensor.matmul(dot_ps[:, 512:N], EL[:],
                             EF2[:, 512:N],
                             start=True, stop=False)
            # rank-1 updates: += 1 x sqf  and  += sql^T x 1
            nc.tensor.matmul(dot_ps[:, 0:512], ONESR[:, 0:ALOC],
                             SQF[:, 0:512],
                             start=False, stop=False)
            nc.tensor.matmul(dot_ps[:, 512:N], ONESR[:, 0:ALOC],
                             SQF[:, 512:N],
                             start=False, stop=False)
            nc.tensor.matmul(dot_ps[:, 0:512], SQLR[:],
                             ONESR[:, 0:512],
                             start=False, stop=True)
            nc.tensor.matmul(dot_ps[:, 512:N], SQLR[:],
                             ONESR[:, 0:128],
                             start=False, stop=True)

            PRE2 = sg.tile([ALOC, N], f32)
            nc.vector.tensor_scalar(out=PRE2[:], in0=dot_ps[:], scalar1=0.0,
                                    scalar2=None, op0=Alu.max)

            # masked v' = sqrt(PRE2 + BIG*(lab_k == lab_a)), bf16 direct
            EQB = sg.tile([ALOC, N], f32)
            nc.vector.tensor_scalar(out=EQB[:], in0=LBC[:], scalar1=LLT[:],
                                    scalar2=BIG, op0=Alu.is_equal,
                                    op1=Alu.mult)
            PREM = sg.tile([ALOC, N], f32)
            nc.vector.tensor_add(PREM[:], PRE2[:], EQB[:])
            VMB = sg.tile([ALOC, N], bf16)
            nc.scalar.activation(out=VMB[:], in_=PREM[:], func=Act.Sqrt)
            vmd = dram.tile([ALOC, N], bf16)
            nc.sync.dma_start(out=vmd[:], in_=VMB[:])

            # ---- vb prefetch: 8 units per tile, one SWDGE DMA per half ----
            # vb8[p, s, k] = vmd[(p >= SPLIT)*UNITS + u0 + s, k]: rows
            # [0,SPLIT) carry side A's v', rows [SPLIT,128) side B's (solo
            # units have both rows identical, so the split is harmless).
            # HWDGE only spreads a DMA across the 16 SDMA engines for
            # multiple-of-16 partition counts; ours are 61/67, so those
            # serialize on one engine. SWDGE (Pool queue) swizzles in
            # software and spreads fine -> issue all vb loads there.
            vb_tiles = {}
            VBU = 8

            def issue_vb(g):
                u0 = VBU * g
                if u0 >= UNITS:
                    return
                nu = min(VBU, UNITS - u0)
                vb8 = vbp.tile([128, VBU, N], bf16, tag="vb", name="vb")
                pitch = VBU * N
                for half, (p0, np_) in enumerate(((0, SPLIT),
                                                  (SPLIT, 128 - SPLIT))):
                    dst = bass.AP(tensor=vb8.tensor,
                                  offset=vb8.offset + p0 * pitch,
                                  ap=[[pitch, np_], [N, nu], [1, N]])
                    src = bass.AP(tensor=vmd.tensor,
                                  offset=vmd.offset + (half * UNITS + u0) * N,
                                  ap=[[0, np_], [N, nu], [1, N]])
                    nc.gpsimd.dma_start(out=dst, in_=src)
                vb_tiles[g] = vb8

            for g in range(PREFETCH):
                issue_vb(g)

            # DIST (f32) for thresholds
            DIST = sg.tile([ALOC, N], f32)
            nc.scalar.activation(out=DIST[:], in_=PRE2[:], func=Act.Sqrt)

            # ---- thresholds: T_sel[p,u] from transposed DIST + masks ----
            TSEL = sg.tile([128, UNITS], f32)
            first = True
            for c in range(NBLK):
                tr_ps = ps_tr.tile([128, ALOC], f32, tag="tr")
                nc.tensor.transpose(tr_ps[:], DIST[:, c * 128:(c + 1) * 128],
                                    ident[0:ALOC, 0:ALOC])
                for side in range(2):
                    t = sg.tile([128, UNITS], f32, tag="tstmp", name="tstmp")
                    nc.vector.scalar_tensor_tensor(
                        out=t[:], in0=tr_ps[:, side * UNITS:
                                            (side + 1) * UNITS],
                        scalar=MARGIN,
                        in1=SELM[:, (2 * c + side) * UNITS:
                                 (2 * c + side + 1) * UNITS],
                        op0=Alu.add, op1=Alu.mult)
                    if first:
                        nc.vector.tensor_copy(TSEL[:], t[:])
                        first = False
                    else:
                        nc.vector.tensor_add(TSEL[:], TSEL[:], t[:])

            # ---- main loop ----
            ACC = sg.tile([128, UNITS], f32)
            q_ps = ps_q.tile([1, N], f32, tag="q", name="q")
            for u in range(UNITS):
                if u % VBU == 0:
                    issue_vb(u // VBU + PREFETCH)
                vb = vb_tiles[u // VBU][:, u % VBU, :]
                tcol = TSEL[:, u:u + 1]
                # compare (non-accumulating, 2x mode)
                mt = mtp.tile([128, N], bf16, tag="mt", name="mt")
                nc.vector.tensor_scalar(out=mt[:], in0=vb, scalar1=tcol,
                                        scalar2=None, op0=Alu.is_lt)
                # count: accumulate into one psum row across all units
                nc.tensor.matmul(q_ps[:, 0:512], onesb[:], mt[:, 0:512],
                                 start=(u == 0), stop=(u == UNITS - 1))
                nc.tensor.matmul(q_ps[:, 512:N], onesb[:], mt[:, 512:N],
                                 start=(u == 0), stop=(u == UNITS - 1))
                # relu-sum pass -> ACC[:, u]
                rcol = ACC[:, u:u + 1]
                if _relu_engine(u) == "A":
                    ja = junkp.tile([128, N], bf16, tag="jA", name="jA")
                    nc.scalar.activation(out=ja[:], in_=vb, func=Act.Relu,
                                         bias=tcol, scale=-1.0,
                                         accum_out=rcol)
                else:
                    jd = junkp.tile([128, N], bf16, tag="jD", name="jD")
                    nc.vector.scalar_tensor_tensor(
                        out=jd[:], in0=vb, scalar=tcol, in1=ZB[:],
                        op0=Alu.subtract, op1=Alu.min, accum_out=rcol)

            # count drain + outputs
            qsb = sg.tile([1, N], f32)
            QOUT = sg.tile([1, 2], f32)
            nc.vector.memset(QOUT[:], 0.0)
            nc.scalar.activation(out=qsb[:], in_=q_ps[:], func=Act.Identity,
                                 bias=0.0, scale=1.0, accum_out=QOUT[:, 0:1])
            nc.sync.dma_start(out=out2_d[:], in_=QOUT[:])
            nc.sync.dma_start(out=out_d[:], in_=ACC[:])

    nc.compile()
    return nc


def _get_program():
    if "nc" not in _CACHE:
        _CACHE["nc"] = _build_program()
    return _CACHE["nc"]


# ---- host-side unit construction (shared by builder consts + decode) ----

def _plan(lab, split=None):
    """Global pairing plan: layouts (labelA, labelB|None) and unit list.
    Side A of a pair must have <= split members, side B <= 128 - split, so
    side B can sit at the fixed partition offset `split` in the shared
    program."""
    import collections
    if split is None:
        split = SPLIT
    cnt = collections.Counter(lab.tolist())
    order = [l for l, _ in sorted(cnt.items(), key=lambda kv: -kv[1])]
    used = set()
    layouts = []          # (labA, labB or None)
    for la in order:
        if la in used:
            continue
        used.add(la)
        lb = None
        for l2 in order:
            if l2 in used:
                continue
            if cnt[la] <= split and cnt[l2] <= 128 - split:
                lb = l2
            elif cnt[l2] <= split and cnt[la] <= 128 - split:
                la, lb = l2, la
            else:
                continue
            used.add(l2)
            break
        layouts.append((la, lb))
    members = {l: np.where(lab == l)[0] for l in cnt}
    units = []            # (layout_idx, anchorA, anchorB or -1)
    for li, (la, lb) in enumerate(layouts):
        ma = members[la]
        mb = members[lb] if lb is not None else np.array([], np.int64)
        npair = min(len(ma), len(mb))
        for i in range(npair):
            units.append((li, int(ma[i]), int(mb[i])))
        big = ma if len(ma) >= len(mb) else mb
        for i in range(npair, len(big)):
            units.append((li, int(big[i]), -1))
    return layouts, members, units


def _configure(lab):
    """Size the per-core program from the actual label distribution; pick
    the block split offset that minimizes the unit count."""
    global UNITS, ALOC, SPLIT
    best = None
    for s in range(32, 97):
        n = len(_plan(lab, s)[2])
        if best is None or n < best[0]:
            best = (n, s)
    u = -(-best[0] // NCORES)
    assert 2 * u <= 128, f"ALOC {2 * u} > 128"
    if u != UNITS or best[1] != SPLIT:
        assert "nc" not in _CACHE, "program already compiled with old config"
        UNITS = u
        ALOC = 2 * u
        SPLIT = best[1]


def _core_layouts(lab):
    layouts, members, units = _plan(lab)
    # pad with dummy units (layout -1)
    units = units + [(-1, -1, -1)] * (NCORES * UNITS - len(units))
    per_core = []
    for r in range(NCORES):
        chunk = units[r * UNITS:(r + 1) * UNITS]
        used = []
        for li, _, _ in chunk:
            if li >= 0 and li not in used:
                used.append(li)
        assert len(used) <= NBLK, f"core {r}: {len(used)} layouts"
        # k-permutation: tile c hosts layout used[c]; side A block at row 0,
        # side B block at row SPLIT of the tile.
        perm = np.full(N, -1, np.int64)
        blocked = []
        for c, li in enumerate(used):
            la, lb = layouts[li]
            ma = members[la]
            perm[128 * c:128 * c + len(ma)] = ma
            blocked.append(ma)
            if lb is not None:
                mb = members[lb]
                perm[128 * c + SPLIT:128 * c + SPLIT + len(mb)] = mb
                blocked.append(mb)
        blk = np.concatenate(blocked) if blocked else np.array([], np.int64)
        filler = np.setdiff1d(np.arange(N), blk)
        perm[perm == -1] = filler
        # anchors + select masks; slot u = side A, slot UNITS+u = side B
        anchors = np.zeros(ALOC, np.int64)
        selmask = np.zeros((128, NBLK, 2, UNITS), np.float32)
        for ui, (li, aa, ab) in enumerate(chunk):
            if li < 0:
                continue
            c = used.index(li)
            la, lb = layouts[li]
            ma = members[la]
            anchors[ui] = aa
            anchors[UNITS + ui] = ab if ab >= 0 else aa
            if ab >= 0:
                mb = members[lb]
                selmask[0:len(ma), c, 0, ui] = 1.0
                selmask[int(np.where(ma == aa)[0][0]), c, 0, ui] = 0.0
                selmask[SPLIT:SPLIT + len(mb), c, 1, ui] = 1.0
                selmask[SPLIT + int(np.where(mb == ab)[0][0]),
                        c, 1, ui] = 0.0
            else:
                in_a = bool(np.isin(aa, ma))
                mown = ma if in_a else members[lb]
                off = 0 if in_a else SPLIT
                selmask[off:off + len(mown), c, 0, ui] = 1.0
                selmask[off + int(np.where(mown == aa)[0][0]),
                        c, 0, ui] = 0.0
        per_core.append((anchors, perm, selmask))
    return per_core


def _make_inputs(embeddings, labels):
    e = np.ascontiguousarray(embeddings.reshape(N, D).astype(np.float32))
    lab = labels.reshape(N).astype(np.int64)
    eT = np.ascontiguousarray(e.T)                        # [D, N]
    labf = lab.astype(np.float32)
    in_maps = []
    for anchors, perm, selmask in _core_layouts(lab):
        in_maps.append({
            "efT2": np.ascontiguousarray(-2.0 * eT[:, perm]),
            "elocT": np.ascontiguousarray(eT[:, anchors]),
            "labrow": np.ascontiguousarray(labf[perm].reshape(1, N)),
            "llocT": np.ascontiguousarray(labf[anchors].reshape(ALOC, 1)),
            "selm": np.ascontiguousarray(
                selmask.reshape(128, NBLK * 2 * UNITS)),
        })
    return in_maps


def _decode(outs, outs2):
    total = 0.0
    count = 0.0
    for r in range(NCORES):
        acc = np.asarray(outs[r]).astype(np.float64)
        for u in range(UNITS):
            s = acc[:, u].sum()
            total += s if _relu_engine(u) == "A" else -s
        count += float(np.asarray(outs2[r]).reshape(-1)[0])
    return total, count


def run_on_device(embeddings: np.ndarray, labels: np.ndarray, **run_kwargs):
    from concourse.bass_utils import run_bass_kernel_spmd
    _configure(np.asarray(labels).reshape(N).astype(np.int64))
    nc = _get_program()
    in_maps = _make_inputs(embeddings, labels)
    res = run_bass_kernel_spmd(nc, in_maps, core_ids=list(range(NCORES)),
                               **run_kwargs)
    total, count = _decode([res.results[r]["out"] for r in range(NCORES)],
                           [res.results[r]["out2"] for r in range(NCORES)])
    return total, count, res


def kernel(embeddings: np.ndarray, labels: np.ndarray):
    embeddings = np.asarray(embeddings)
    labels = np.asarray(labels)
    total, count, _ = run_on_device(embeddings, labels)

    lab = np.asarray(labels).reshape(-1)
    cnt = np.bincount(lab.astype(np.int64), minlength=1)
    per = cnt[lab.astype(np.int64)]
    num_valid = int(((per - 1) * (N - per)).sum())

    nv = np.float32(num_valid)
    ne = np.float32(count)
    tot = np.float32(total)
    if ne > 0:
        loss = np.float32(tot / np.maximum(ne, np.float32(1.0)))
    else:
        loss = np.float32(0.0)
    frac = np.float32(ne / (nv + np.float32(1e-16)))
    return (np.array(loss, np.float32), np.array(nv, np.float32),
            np.array(ne, np.float32), np.array(frac, np.float32))

